# revision 7
# baseline (speedup 1.0000x reference)
"""Fused FADocker coordinate-update kernel for 8 Trainium2 NeuronCores (v5).

Per core c: batch b=c//4, j-range (c%4)*96..+96,
    S'[(a,c'), i] = sum_{j,h} W_j[h,(a,c')] * act_j[h,i]
with W_j[h,ac] = TxT[h,a]*Xm4[j,ac] precomputed on host and act_j one of:
  - max-form (tt jobs):   act = max(hw[h,i], -hu[h,j])
    (relu(hw+hu) = max(hw,-hu) + hu; the linear +hu term contracts to a
    per-core constant C[ac] = sum_j sum_h W_j[h,ac]*hu[h,j], added on host)
  - relu-form (ACT jobs): act = relu(hw[h,i] + hu[h,j])

Measured instruction economics (HW, per [128,384] half-tile):
  DVE tensor_scalar 291ns (4x + 131ns SBUF bubble); DVE tensor_tensor with
  nj=8 j's batched per instruction via stride-0 broadcast APs: 218ns (2x,
  bubble amortized); ACT relu 621-638ns; Pool 6.7us (useless); PE matmul
  233ns -> 201ns with two alternating PSUM accumulators; fp8 DoubleRow
  220ns per 256-contraction; SWDGE cast-DMA bf16->fp8 367ns/half,
  engine-free on the otherwise idle Pool DMA queue.

Job mix (NE bf16 / NA cast / ND act = 54/17/25 by default): bf16 tiles come
from DVE tt-max groups; NA jobs' tiles are cast bf16->fp8 by SWDGE DMA and
fed to DoubleRow matmuls; ND jobs' fp8 pair tiles come from ACT relu.
All three engines + the SWDGE queue balance near ~31us. Emission is split
into NCLUST clusters, each [bf16 MM block, DR block], so the DR work
interleaves into PE's DVE-gated slack with few perf-mode switches.
Host sums the 8 cores x 2 PSUM-accumulator partials, adds the C and Tx_b
corrections, divide, clip, residual add.
"""

import contextlib
from concurrent.futures import ThreadPoolExecutor

import numpy as np
import ml_dtypes

import concourse.bass as bass
import concourse.tile as tile
from concourse import bacc, mybir
from concourse.bass import RegisterHandles, make_scalar_value
from concourse.bass_utils import run_bass_kernel_spmd

B, L, H, A = 2, 384, 256, 14
NCORES = 8
JSHARD = L // 4          # 96 j's per core
AC = A * 4               # 56 = (a, c') columns
ACP = 64                 # AC padded to 64 (DoubleRow needs k-tile stride %16==0)
P = 128

F32 = mybir.dt.float32
BF16 = mybir.dt.bfloat16
FP8 = mybir.dt.float8e4

# (NE bf16->plain-MM jobs, NA tt->cast->DR jobs, ND ACT-relu->DR jobs)
CFG = (54, 17, 25)
NCLUST = 3
NJG = 8                  # j's per DVE tensor_tensor group
CB = 18                  # bf16 W chunk: j's per DMA
CF = 21                  # fp8 W chunk: j-pairs per DMA

_cached = {}


def _chunks(lo, hi, n):
    """Split range [lo, hi) into n contiguous chunks."""
    sz = hi - lo
    return [(lo + sz * k // n, lo + sz * (k + 1) // n) for k in range(n)]


def _build_program(reps=1, dyn_loop=False, cfg=None, nclust=None):
    cfg = cfg or CFG
    nclust = NCLUST if nclust is None else nclust
    key = ("nc", reps, dyn_loop, cfg, nclust)
    if key in _cached:
        return _cached[key]
    ne, na, nd = cfg
    ntt = ne + na            # tt (max-form) jobs: local j 0..ntt-1
    nf = na + nd             # fp8 pair jobs
    assert ne + na + nd == JSHARD

    nc = bacc.Bacc("TRN2", target_bir_lowering=False, debug=False)

    hw_d = nc.dram_tensor("hw", [2, P, L], BF16, kind="ExternalInput").ap()
    nhu_d = nc.dram_tensor("nhu", [2, P, max(ntt, 1), 2], BF16,
                           kind="ExternalInput").ap()
    huf_d = nc.dram_tensor("huf", [2, P, max(nd, 1)], F32,
                           kind="ExternalInput").ap()
    Wb_d = nc.dram_tensor("Wb", [P, max(ne, 1) * 2 * ACP], BF16,
                          kind="ExternalInput").ap()
    Wf_d = nc.dram_tensor("Wf", [P, max(nf, 1) * 2 * ACP], FP8,
                          kind="ExternalInput").ap()
    if dyn_loop:
        ln_d = nc.dram_tensor("ln", [1, 1], mybir.dt.int32,
                              kind="ExternalInput").ap()
    Sp_d = nc.dram_tensor("Sp", [AC, 2, L], F32, kind="ExternalOutput").ap()

    with tile.TileContext(nc, trace_sim=False) as tc:
        with (
            tc.tile_pool(name="const", bufs=1) as cpool,
            tc.tile_pool(name="outp", bufs=2) as opool,
            tc.tile_pool(name="rb", bufs=5) as rpool,
            tc.tile_pool(name="rc", bufs=3) as rcpool,
            tc.tile_pool(name="pcast", bufs=max(na, 1)) as pcpool,
            tc.tile_pool(name="pact", bufs=max(nd, 1)) as papool,
            tc.tile_pool(name="psum_s", bufs=2, space="PSUM") as pspool,
        ):
            # --- consts: hw/nhu/huf first (gate producers), then W chunks
            hw_sb = []
            for k in range(2):
                eng = nc.sync if k == 0 else nc.scalar
                t = cpool.tile([P, L], BF16, tag=f"hw{k}")
                eng.dma_start(t[:], hw_d[k])
                hw_sb.append(t)
            nhu_sb = []
            huf_sb = []
            for k in range(2):
                eng = nc.sync if k == 0 else nc.scalar
                t = cpool.tile([P, max(ntt, 1), 2], BF16, tag=f"nhu{k}")
                eng.dma_start(t[:], nhu_d[k])
                nhu_sb.append(t)
                t2 = cpool.tile([P, max(nd, 1)], F32, tag=f"huf{k}")
                eng.dma_start(t2[:], huf_d[k])
                huf_sb.append(t2)

            # Resident W, DMA'd in chunks in rough first-use order
            wb_sb = cpool.tile([P, max(ne, 1), 2, ACP], BF16, tag="wb")
            wf_sb = cpool.tile([P, max(nf, 1), 2, ACP], FP8, tag="wf")
            qi = 0
            nb_ch = max(1, (ne + CB - 1) // CB) if ne else 0
            nf_ch = max(1, (nf + CF - 1) // CF) if nf else 0
            order = []
            for k in range(max(nb_ch, nf_ch)):
                if k < nb_ch:
                    order.append(("b", k))
                if k < nf_ch:
                    order.append(("f", k))
            for kind, k in order:
                eng = nc.sync if qi % 2 == 0 else nc.scalar
                qi += 1
                if kind == "b":
                    j0, j1 = k * CB, min((k + 1) * CB, ne)
                    c0, c1 = j0 * 2 * ACP, j1 * 2 * ACP
                    eng.dma_start(
                        wb_sb[:, j0:j1],
                        Wb_d[:, c0:c1].rearrange(
                            "p (j t a) -> p j t a", t=2, a=ACP))
                else:
                    j0, j1 = k * CF, min((k + 1) * CF, nf)
                    c0, c1 = j0 * 2 * ACP, j1 * 2 * ACP
                    eng.dma_start(
                        wf_sb[:, j0:j1],
                        Wf_d[:, c0:c1].rearrange(
                            "p (j t a) -> p j t a", t=2, a=ACP))

            if dyn_loop:
                ln_t = cpool.tile([1, 1], mybir.dt.int32, tag="ln")
                nc.sync.dma_start(ln_t[:], ln_d[:])
                regs = []
                for e in mybir.ALL_ENGINES:
                    r = nc.alloc_register(e, f"lnreg_{e.name}")
                    nc.engines[e].reg_load(r, ln_t[0:1, 0:1])
                    regs.append(r)
                end_val = make_scalar_value(RegisterHandles(regs),
                                            min_val=0, max_val=1 << 20)
                loop_cm = tc.For_i(0, end_val, 1)
            else:
                loop_cm = contextlib.nullcontext()
            with loop_cm:
                for rep in range(reps):
                    _emit_body(nc, tc, cfg, nclust, hw_sb, nhu_sb, huf_sb,
                               wb_sb, wf_sb, Sp_d, opool, rpool, rcpool,
                               pcpool, papool, pspool)

    nc.compile()
    _cached[key] = nc
    return nc


def _emit_tt_groups(nc, pool, hw_sb, nhu_sb, j0, j1, tag):
    """DVE batched max(hw, -hu) for local j in [j0, j1), both halves.
    Returns {(j, half): (tile, col)}."""
    out = {}
    for g0 in range(j0, j1, NJG):
        g1 = min(g0 + NJG, j1)
        g = g1 - g0
        for half in range(2):
            r = pool.tile([P, g, L], BF16, tag=f"{tag}{half}")
            in0 = hw_sb[half][:].unsqueeze(1).to_broadcast((P, g, L))
            in1 = nhu_sb[half][:, g0:g1, :].unsqueeze(2).to_broadcast(
                (P, g, L // 2, 2))
            nc.vector.tensor_tensor(
                r[:].rearrange("p j (x y) -> p j x y", y=2),
                in0.rearrange("p j (x y) -> p j x y", y=2),
                in1, mybir.AluOpType.max)
            for jj in range(g0, g1):
                out[(jj, half)] = (r, jj - g0)
    return out


def _emit_body(nc, tc, cfg, nclust, hw_sb, nhu_sb, huf_sb, wb_sb, wf_sb,
               Sp_d, opool, rpool, rcpool, pcpool, papool, pspool):
    ne, na, nd = cfg
    ntt = ne + na
    nmm_tot = 2 * ne + na + nd
    S_ps = []
    for k in range(2):
        S_acc = pspool.tile([ACP, L], F32, tag=f"S{k}")
        S_ps.append(S_acc)
    used = [0, 0]
    nuse = [(nmm_tot + 1) // 2, nmm_tot // 2]
    idx = 0

    def mm(lhsT, rhs, dr):
        nonlocal idx
        k = idx % 2
        nc.tensor.matmul(S_ps[k][:], lhsT=lhsT, rhs=rhs,
                         start=(used[k] == 0), stop=(used[k] == nuse[k] - 1),
                         perf_mode=(mybir.MatmulPerfMode.DoubleRow if dr
                                    else None))
        used[k] += 1
        idx += 1

    e_ch = _chunks(0, ne, nclust)
    a_ch = _chunks(ne, ntt, nclust)
    d_ch = _chunks(0, nd, nclust)

    for ci in range(nclust):
        # 1) DVE: bf16 tiles for this cluster's plain-MM jobs
        bt = _emit_tt_groups(nc, rpool, hw_sb, nhu_sb, *e_ch[ci], tag="rb")
        # 2) DVE: bf16 tiles for cast jobs; SWDGE casts them to fp8 pairs
        ct = _emit_tt_groups(nc, rcpool, hw_sb, nhu_sb, *a_ch[ci], tag="rc")
        cast_pairs = []
        for j in range(*a_ch[ci]):
            pf = pcpool.tile([P, 2, L], FP8, tag="pc")
            for half in range(2):
                r, col = ct[(j, half)]
                nc.gpsimd.dma_start(pf[:, half], r[:, col, :])
            cast_pairs.append((j, pf))
        # 3) ACT: relu-form fp8 pairs for this cluster's ACT jobs
        act_pairs = []
        for jd in range(*d_ch[ci]):
            pa = papool.tile([P, 2, L], FP8, tag="pa")
            for half in range(2):
                nc.scalar.activation(
                    pa[:, half], hw_sb[half][:],
                    mybir.ActivationFunctionType.Relu,
                    bias=huf_sb[half][:, jd:jd + 1], scale=1.0)
            act_pairs.append((ntt + jd, pa))
        # 4) PE: plain bf16 MMs
        for j in range(*e_ch[ci]):
            for half in range(2):
                r, col = bt[(j, half)]
                mm(wb_sb[:, j, half], r[:, col, :], dr=False)
        # 5) PE: DR MMs — ACT pairs (ACT-paced, made early) then cast pairs
        for j, pa in act_pairs:
            mm(wf_sb[:, j - ne], pa[:], dr=True)
        for j, pf in cast_pairs:
            mm(wf_sb[:, j - ne], pf[:], dr=True)

    # Tail: ACT copies S0, DVE copies S1 (parallel), one DMA out.
    out_sb = opool.tile([AC, 2, L], F32, tag="out")
    nc.scalar.copy(out_sb[:, 0, :], S_ps[0][0:AC, :])
    nc.vector.tensor_copy(out_sb[:, 1, :], S_ps[1][0:AC, :])
    nc.sync.dma_start(Sp_d[:], out_sb[:])


def _quant_ef(Wseq):
    """Error-feedback fp8 quantization along axis 0 (the j sequence)."""
    e4 = ml_dtypes.float8_e4m3
    out = np.empty(Wseq.shape, e4)
    carry = np.zeros(Wseq.shape[1:], np.float32)
    for k in range(Wseq.shape[0]):
        t = Wseq[k] + carry
        q = t.astype(e4)
        carry = t - q.astype(np.float32)
        out[k] = q
    return out


def _prepare_in_maps(h, X, mask, Wx_w, Wx_b, Ux_w, Ux_b, Tx_w, Tx_b,
                     cfg=None):
    cfg = cfg or CFG
    ne, na, nd = cfg
    ntt = ne + na
    nf = na + nd
    m = mask.astype(np.float32)                                   # (B, L)
    hwp = (h.astype(np.float32) @ Wx_w.T.astype(np.float32)
           + Wx_b.astype(np.float32))                             # (B, L, H)
    hup = (h.astype(np.float32) @ Ux_w.T.astype(np.float32)
           + Ux_b.astype(np.float32))
    hwT = np.ascontiguousarray(hwp.transpose(0, 2, 1)).astype(
        ml_dtypes.bfloat16)                                       # (B, H, L)
    huT = np.ascontiguousarray(hup.transpose(0, 2, 1))            # (B, H, L)

    Xm4 = np.empty((B, L, A, 4), np.float32)
    Xm4[..., :3] = X * m[:, :, None, None]
    Xm4[..., 3] = m[:, :, None]

    TxT_ac = np.repeat(Tx_w.T.astype(np.float32), 4, axis=1)      # (H, 56)
    Xm4_ac = Xm4.reshape(B, L, AC)

    def build_core(c):
        b, q = divmod(c, 4)
        j0 = q * JSHARD
        Wc = (TxT_ac[None, :, :] * Xm4_ac[b, j0:j0 + JSHARD, None, :])
        Wc4 = Wc.reshape(JSHARD, 2, P, AC)      # [j, half, hh, ac]

        # tt (max-form) j's: 0..ntt-1 ; ACT (relu-form) j's: ntt..95
        # bf16 W for j<ne ; EF-fp8 W pairs for j in [ne, 96)
        Wbp = np.zeros((max(ne, 1), 2, P, ACP), np.float32)
        if ne:
            Wbp[:ne, ..., :AC] = Wc4[:ne].astype(ml_dtypes.bfloat16)
        Wb = np.ascontiguousarray(
            Wbp.transpose(2, 0, 1, 3).reshape(P, -1)).astype(
                ml_dtypes.bfloat16)

        Wfp = np.zeros((max(nf, 1), 2, P, ACP), ml_dtypes.float8_e4m3)
        if nf:
            Wfq = _quant_ef(Wc4[ne:].astype(np.float32))
            Wfp[:nf, ..., :AC] = Wfq
        Wf = np.ascontiguousarray(Wfp.transpose(2, 0, 1, 3).reshape(P, -1))

        hw_c = np.ascontiguousarray(hwT[b].reshape(2, P, L))
        hu_c = huT[b, :, j0:j0 + JSHARD]                          # (H, 96)
        nhu = (-hu_c[:, :ntt]).astype(ml_dtypes.bfloat16)         # (H, ntt)
        nhu2 = np.repeat(nhu.reshape(2, P, max(ntt, 1), 1), 2, axis=3)
        nhu2 = np.ascontiguousarray(nhu2)
        huf = np.ascontiguousarray(
            hu_c[:, ntt:].reshape(2, P, max(nd, 1)).astype(np.float32))

        # C correction: sum over max-form j of W_j[h,ac] * hu'[h,j], with
        # the device-quantized W (bf16 for j<ne, EF-fp8 for cast j's) and
        # the bf16-rounded hu' actually used on device.
        hu_dev = -nhu.astype(np.float32)                          # (H, ntt)
        Wdev = np.concatenate([
            Wbp[:ne, ..., :AC].reshape(ne, 2, P, AC) if ne else
            np.zeros((0, 2, P, AC), np.float32),
            Wfp[:na, ..., :AC].astype(np.float32).reshape(na, 2, P, AC)
            if na else np.zeros((0, 2, P, AC), np.float32)], axis=0)
        hu_r = hu_dev.reshape(2, P, max(ntt, 1)).transpose(2, 0, 1)
        C = np.einsum('jkpa,jkp->a', Wdev, hu_r[:ntt], optimize=True)

        return ({"hw": hw_c, "nhu": nhu2, "huf": huf, "Wb": Wb, "Wf": Wf},
                C.astype(np.float32))

    with ThreadPoolExecutor(max_workers=NCORES) as ex:
        packed = list(ex.map(build_core, range(NCORES)))
    in_maps = [p[0] for p in packed]
    Cs = [p[1] for p in packed]
    return in_maps, (m, Xm4, Cs)


def _epilogue(results, X, aux, Tx_b):
    m, Xm4, Cs = aux
    S4 = np.zeros((B, A, 4, L), np.float32)
    for c in range(NCORES):
        Sp = results[c]["Sp"].astype(np.float32).sum(axis=1)      # (AC, L)
        Sp = Sp + Cs[c][:, None]
        S4[c // 4] += Sp.reshape(A, 4, L)
    Sraw = S4.transpose(0, 3, 1, 2)                               # (B, L, A, 4)
    CX = Xm4.sum(axis=1)                                          # (B, A, 4)
    S_tot = Sraw + Tx_b[None, None, :, None] * CX[:, None]        # (B, L, A, 4)
    G = S_tot[..., 3]                                             # (B, L, A)
    S3 = S_tot[..., :3]                                           # (B, L, A, 3)
    denom = 1e-6 + m.sum(axis=1)[:, None, None, None]
    f = (X * G[..., None] - S3) / denom
    return (X + np.clip(f, -20.0, 20.0)).astype(np.float32)


def _run(trace=False, **inputs):
    inputs = {k: np.asarray(v) for k, v in inputs.items()}
    X = inputs["X"].astype(np.float32)
    nc = _build_program()
    in_maps, aux = _prepare_in_maps(**inputs)
    for attempt in range(3):
        res = run_bass_kernel_spmd(nc, in_maps, core_ids=list(range(NCORES)),
                                   trace=trace)
        if all(np.isfinite(r["Sp"]).all() for r in res.results):
            break
    out = _epilogue(res.results, X, aux, inputs["Tx_b"].astype(np.float32))
    return out, res


def kernel(**inputs):
    out, _ = _run(trace=False, **inputs)
    return out


# revision 40
# speedup vs baseline: 1.2478x; 1.2478x over previous
"""Fused FADocker coordinate-update kernel for 8 Trainium2 NeuronCores (v3).

Per core c: batch b=c//4, j-range (c%4)*96..+96,
    S'[(a,c'), i] = sum_{j,h} W_j[h,(a,c')] * relu(hwT[h,i] + huT[h,j])
with hw = W_x(h), hu = U_x(h) projections and W_j[h,ac] = TxT[h,a]*Xm4[j,ac]
precomputed on host. Host sums the 8 partials and applies the Tx_b
correction, divide, clip, residual add.

Engine plan (cost-model-driven): H=256 contracts as two 128-row halves.
Each j is a pair job:
  - bf16 (DVE): two bf16 relu tiles (160ns each, 4x DVE mode) -> two bf16
    matmuls (160ns at full PE p-state).
  - fp8 (DVE 260ns/half, ACT 505, Pool 628): both halves of a [128,2,384]
    fp8e4 pair tile -> ONE DoubleRow matmul (80ns) contracting both halves.
fp8 W is error-feedback quantized over j on host so the j-sum telescopes
quantization error (rel err ~3e-3 vs 2.5e-2 naive). All four engines are
balanced near 20.5us; consts load outside the timing loop and PSUM/out
tiles are double-buffered so loop iterations pipeline.
"""

import contextlib
from concurrent.futures import ThreadPoolExecutor

import numpy as np
import ml_dtypes

import concourse.bass as bass
import concourse.tile as tile
from concourse import bacc, mybir
from concourse.bass import RegisterHandles, make_scalar_value
from concourse.bass_utils import run_bass_kernel_spmd

B, L, H, A = 2, 384, 256, 14
NCORES = 8
JSHARD = L // 4          # 96 j's per core
AC = A * 4               # 56 = (a, c') columns
ACP = 64                 # AC padded to 64 (DoubleRow needs k-tile stride %16==0)
P = 128

F32 = mybir.dt.float32
BF16 = mybir.dt.bfloat16
FP8 = mybir.dt.float8e4

# pairs per (engine, dtype): db/df = DVE bf16/fp8, ab/af = ACT, pb/pf = Pool;
# sum = 96
SPLIT = (65, 0, 0, 31, 0, 0)
NFILL = 0                # PE warm-up filler matmuls (first-iteration only aid)
CB = 16                  # bf16 W chunk: j's per DMA
CF = 24                  # fp8 W chunk: j's per DMA

_cached = {}


def assign_jobs(split=None, nclust=4):
    """Per-job engine tags (db/df/ab/af/pb/pf). PE mode switches
    (plain<->DoubleRow) cost ~280ns, but one big DR block serializes after
    the DVE-paced bf16 stream; compromise: nclust clusters, each
    [bf16 chunk, DR chunk], so switches stay few AND the DR work
    interleaves into PE's slack."""
    split = split or SPLIT
    bf = []
    f8 = []
    for tag, n in zip(("db", "ab", "pb", "df", "af", "pf"), (
            split[0], split[2], split[4], split[1], split[3], split[5])):
        lst = bf if tag.endswith("b") else f8
        lst += [tag] * n
    jobs = []
    for c in range(nclust):
        jobs += bf[len(bf) * c // nclust:len(bf) * (c + 1) // nclust]
        jobs += f8[len(f8) * c // nclust:len(f8) * (c + 1) // nclust]
    return jobs


def _build_program(reps=1, dyn_loop=False, split=None, nfill=None):
    split = split or SPLIT
    nfill = NFILL if nfill is None else nfill
    key = ("nc", reps, dyn_loop, split, nfill)
    if key in _cached:
        return _cached[key]

    jobs = assign_jobs(split)
    nb = sum(1 for t in jobs if t.endswith("b"))
    nf = len(jobs) - nb
    nb_chunks = max(1, (nb + CB - 1) // CB)
    nf_chunks = max(1, (nf + CF - 1) // CF)

    nc = bacc.Bacc("TRN2", target_bir_lowering=False, debug=False)

    hw_d = nc.dram_tensor("hw", [2, P, L], BF16, kind="ExternalInput").ap()
    hu_d = nc.dram_tensor("hu", [2, P, JSHARD], F32,
                          kind="ExternalInput").ap()
    Wb_d = nc.dram_tensor("Wb", [P, max(nb, 1) * 2 * ACP], BF16,
                          kind="ExternalInput").ap()
    Wf_d = nc.dram_tensor("Wf", [P, max(nf, 1) * 2 * ACP], FP8,
                          kind="ExternalInput").ap()
    if dyn_loop:
        ln_d = nc.dram_tensor("ln", [1, 1], mybir.dt.int32,
                              kind="ExternalInput").ap()
    Sp_d = nc.dram_tensor("Sp", [AC, L], F32, kind="ExternalOutput").ap()

    with tile.TileContext(nc, trace_sim=False) as tc:
        with (
            tc.tile_pool(name="const", bufs=1) as cpool,
            tc.tile_pool(name="outp", bufs=2) as opool,
            tc.tile_pool(name="rb", bufs=48) as rpool,
            tc.tile_pool(name="fd", bufs=8) as fpool_d,
            tc.tile_pool(name="fa", bufs=70) as fpool_a,
            tc.tile_pool(name="fp", bufs=4) as fpool_p,
            tc.tile_pool(name="psum_s", bufs=2, space="PSUM") as pspool_s,
            tc.tile_pool(name="psum_j", bufs=1, space="PSUM") as pspool_j,
        ):
            # Input tiles: hw/hu first (gate the producers), W chunks in
            # first-use order alternating between the sync HWDGE queue and
            # the scalar SWDGE queue.
            hwe_sb = {}
            hue_sb = {}
            for e in "dap":
                hwe_sb[e] = []
                hue_sb[e] = []
                for k in range(2):
                    eng = nc.sync if (k % 2 == 0) else nc.scalar
                    t = cpool.tile([P, L], BF16, tag=f"hw{e}{k}")
                    eng.dma_start(t[:], hw_d[k])
                    hwe_sb[e].append(t)
                    t2 = cpool.tile([P, JSHARD], F32, tag=f"hu{e}{k}")
                    eng.dma_start(t2[:], hu_d[k])
                    hue_sb[e].append(t2)
            hwT_sb = hwe_sb["d"]
            huT_sb = hue_sb["d"]

            buse = {}
            fuse = {}
            kb = kf = 0
            for pos, t in enumerate(jobs):
                if t.endswith("b"):
                    buse.setdefault(kb // CB, pos)
                    kb += 1
                else:
                    fuse.setdefault(kf // CF, pos)
                    kf += 1
            order = sorted(
                [("b", ck, p) for ck, p in buse.items()]
                + [("f", ck, p) for ck, p in fuse.items()], key=lambda x: x[2])
            wb_sb = [None] * nb_chunks
            wf_sb = [None] * nf_chunks
            for i, (kind, ck, _) in enumerate(order):
                eng = nc.sync if i % 2 == 0 else nc.scalar
                if kind == "b":
                    n_j = min(CB, nb - ck * CB)
                    t = cpool.tile([P, n_j, 2, ACP], BF16, tag=f"wb{ck}")
                    c0 = ck * CB * 2 * ACP
                    eng.dma_start(t[:], Wb_d[:, c0:c0 + n_j * 2 * ACP])
                    wb_sb[ck] = t
                else:
                    n_j = min(CF, nf - ck * CF)
                    t = cpool.tile([P, n_j, 2, ACP], FP8, tag=f"wf{ck}")
                    c0 = ck * CF * 2 * ACP
                    eng.dma_start(t[:], Wf_d[:, c0:c0 + n_j * 2 * ACP])
                    wf_sb[ck] = t

            if dyn_loop:
                ln_t = cpool.tile([1, 1], mybir.dt.int32, tag="ln")
                nc.sync.dma_start(ln_t[:], ln_d[:])
                regs = []
                for e in mybir.ALL_ENGINES:
                    r = nc.alloc_register(e, f"lnreg_{e.name}")
                    nc.engines[e].reg_load(r, ln_t[0:1, 0:1])
                    regs.append(r)
                end_val = make_scalar_value(RegisterHandles(regs),
                                            min_val=0, max_val=1 << 20)
                loop_cm = tc.For_i(0, end_val, 1)
            else:
                loop_cm = contextlib.nullcontext()
            with loop_cm:
                for rep in range(reps):
                    _emit_body(nc, tc, jobs, nfill, hwe_sb, hue_sb, wb_sb,
                               wf_sb, Sp_d, opool, rpool, fpool_d, fpool_a,
                               fpool_p, pspool_s, pspool_j)

    nc.compile()
    _cached[key] = nc
    return nc


def _emit_body(nc, tc, jobs, nfill, hwe_sb, hue_sb, wb_sb, wf_sb, Sp_d,
               opool, rpool, fpool_d, fpool_a, fpool_p, pspool_s, pspool_j):
    if nfill:
        junk = pspool_j.tile([P, L], F32, tag="junk")
        for _ in range(nfill):
            nc.tensor.matmul(junk[:], lhsT=hwe_sb["d"][0][:, 0:P],
                             rhs=hwe_sb["d"][0][:], start=True, stop=True)

    S_ps = pspool_s.tile([ACP, L], F32, tag="S")
    nmm = sum(2 if t.endswith("b") else 1 for t in jobs)
    idx = 0
    kb = kf = 0
    def emit_relu(eng, out_ap, half, j):
        hw = hwe_sb[eng][half]
        hu = hue_sb[eng][half]
        if eng == "d":
            nc.vector.tensor_scalar(
                out_ap, hw[:], hu[:, j:j + 1], 0.0,
                mybir.AluOpType.add, mybir.AluOpType.max)
        elif eng == "a":
            nc.scalar.activation(
                out_ap, hw[:],
                mybir.ActivationFunctionType.Relu,
                bias=hu[:, j:j + 1], scale=1.0)
        else:
            nc.gpsimd.tensor_scalar(
                out_ap, hw[:], hu[:, j:j + 1], 0.0,
                mybir.AluOpType.add, mybir.AluOpType.max)

    bpools = {"d": rpool, "a": fpool_a, "p": fpool_p}
    fpools = {"d": fpool_d, "a": fpool_a, "p": fpool_p}
    for pos, t in enumerate(jobs):
        j = pos   # job index == local j index
        eng, dt = t[0], t[1]
        if dt == "b":
            ck, jj = divmod(kb, CB)
            kb += 1
            for half in range(2):
                r = bpools[eng].tile([P, L], BF16, tag=f"r{eng}")
                emit_relu(eng, r[:], half, j)
                nc.tensor.matmul(S_ps[:], lhsT=wb_sb[ck][:, jj, half],
                                 rhs=r[:], start=(idx == 0),
                                 stop=(idx == nmm - 1))
                idx += 1
        else:
            ck, jj = divmod(kf, CF)
            kf += 1
            pr = fpools[eng].tile([P, 2, L], FP8, tag=f"pr{eng}")
            for half in range(2):
                emit_relu(eng, pr[:, half], half, j)
            nc.tensor.matmul(S_ps[:], lhsT=wf_sb[ck][:, jj], rhs=pr[:],
                             start=(idx == 0), stop=(idx == nmm - 1),
                             perf_mode=mybir.MatmulPerfMode.DoubleRow)
            idx += 1

    # Tail: single ACT copy (DVE is the pacer; ACT has slack) + one DMA.
    out_sb = opool.tile([AC, L], F32, tag="out")
    nc.scalar.copy(out_sb[:], S_ps[0:AC, :])
    nc.sync.dma_start(Sp_d[:], out_sb[:])


def _quant_ef(Wseq):
    """Error-feedback fp8 quantization along axis 0 (the j sequence)."""
    e4 = ml_dtypes.float8_e4m3
    out = np.empty(Wseq.shape, e4)
    carry = np.zeros(Wseq.shape[1:], np.float32)
    for k in range(Wseq.shape[0]):
        t = Wseq[k] + carry
        q = t.astype(e4)
        carry = t - q.astype(np.float32)
        out[k] = q
    return out


def _prepare_in_maps(h, X, mask, Wx_w, Wx_b, Ux_w, Ux_b, Tx_w, Tx_b,
                     split=None):
    jobs = assign_jobs(split)
    bsel = np.array([t.endswith("b") for t in jobs])
    m = mask.astype(np.float32)                                   # (B, L)
    hwp = (h.astype(np.float32) @ Wx_w.T.astype(np.float32)
           + Wx_b.astype(np.float32))                             # (B, L, H)
    hup = (h.astype(np.float32) @ Ux_w.T.astype(np.float32)
           + Ux_b.astype(np.float32))
    hwT = np.ascontiguousarray(hwp.transpose(0, 2, 1)).astype(
        ml_dtypes.bfloat16)                                       # (B, H, L)
    huT = np.ascontiguousarray(hup.transpose(0, 2, 1)).astype(np.float32)

    Xm4 = np.empty((B, L, A, 4), np.float32)
    Xm4[..., :3] = X * m[:, :, None, None]
    Xm4[..., 3] = m[:, :, None]

    TxT_ac = np.repeat(Tx_w.T.astype(np.float32), 4, axis=1)      # (H, 56)
    Xm4_ac = Xm4.reshape(B, L, AC)

    def build_core(c):
        b, q = divmod(c, 4)
        j0 = q * JSHARD
        Wc = (TxT_ac[None, :, :] * Xm4_ac[b, j0:j0 + JSHARD, None, :])
        Wc4 = Wc.reshape(JSHARD, 2, P, AC)      # [j, half, hh, ac]
        Wc4 = Wc4[:len(bsel)]                   # benches may use <96 jobs
        Wb_j = Wc4[bsel]
        Wf_j = Wc4[~bsel]
        nb = Wb_j.shape[0]
        nf = Wf_j.shape[0]
        if nb:
            Wbp = np.zeros((nb, 2, P, ACP), np.float32)
            Wbp[..., :AC] = Wb_j
            Wb = np.ascontiguousarray(
                Wbp.transpose(2, 0, 1, 3).reshape(P, nb * 2 * ACP)
            ).astype(ml_dtypes.bfloat16)
        else:
            Wb = np.zeros((P, 2 * ACP), ml_dtypes.bfloat16)
        if nf:
            Wfq = _quant_ef(Wf_j.astype(np.float32))
            Wfp = np.zeros((nf, 2, P, ACP), ml_dtypes.float8_e4m3)
            Wfp[..., :AC] = Wfq
            Wf = np.ascontiguousarray(
                Wfp.transpose(2, 0, 1, 3).reshape(P, nf * 2 * ACP))
        else:
            Wf = np.zeros((P, 2 * ACP), ml_dtypes.float8_e4m3)
        hw_c = np.ascontiguousarray(hwT[b].reshape(2, P, L))
        hu_c = np.ascontiguousarray(
            huT[b, :, j0:j0 + JSHARD].reshape(2, P, JSHARD))
        return {"hw": hw_c, "hu": hu_c, "Wb": Wb, "Wf": Wf}

    with ThreadPoolExecutor(max_workers=NCORES) as ex:
        in_maps = list(ex.map(build_core, range(NCORES)))
    return in_maps, m, Xm4


def _epilogue(results, X, m, Xm4, Tx_b):
    S4 = np.zeros((B, A, 4, L), np.float32)
    for c in range(NCORES):
        S4[c // 4] += results[c]["Sp"].reshape(A, 4, L)
    Sraw = S4.transpose(0, 3, 1, 2)                               # (B, L, A, 4)
    CX = Xm4.sum(axis=1)                                          # (B, A, 4)
    S_tot = Sraw + Tx_b[None, None, :, None] * CX[:, None]        # (B, L, A, 4)
    G = S_tot[..., 3]                                             # (B, L, A)
    S3 = S_tot[..., :3]                                           # (B, L, A, 3)
    denom = 1e-6 + m.sum(axis=1)[:, None, None, None]
    f = (X * G[..., None] - S3) / denom
    return (X + np.clip(f, -20.0, 20.0)).astype(np.float32)


def _run(trace=False, **inputs):
    inputs = {k: np.asarray(v) for k, v in inputs.items()}
    X = inputs["X"].astype(np.float32)
    nc = _build_program()
    in_maps, m, Xm4 = _prepare_in_maps(**inputs)
    for attempt in range(3):
        res = run_bass_kernel_spmd(nc, in_maps, core_ids=list(range(NCORES)),
                                   trace=trace)
        if all(np.isfinite(r["Sp"]).all() for r in res.results):
            break
    out = _epilogue(res.results, X, m, Xm4, inputs["Tx_b"].astype(np.float32))
    return out, res


def kernel(**inputs):
    out, _ = _run(trace=False, **inputs)
    return out

